# revision 9
# baseline (speedup 1.0000x reference)
"""Trainium2 Bass kernel for multi-head attention (b=4, n=2048, d=512, h=8, dk=dv=64).

Sharding: 8 cores = 4 batches x 2 query-halves. Each core computes K/V for its
full batch sequence (2048) and attention outputs for its 1024 query rows.
No collectives needed; host stacks the per-core [1024, 512] outputs.

Per-core dataflow (f32r = TF32-like fast fp32 matmul mode; PV in bf16):
  x^T [512, 2048] staged in SBUF.
  Q^T per head, replicated to both partition halves: qt2 [128, h, i]
  K^T per head stored block-diagonally per j-chunk: kt_bd [128, h, jc, 128]
    rows 0:64  = K^T dims x even 64-j half   (cols 0:64)
    rows 64:128= K^T dims x odd 64-j half    (cols 64:128), zeros elsewhere
  -> S^T matmul has K=128 (full PE rate): out rows = 128 consecutive j's.
  V   = x Wv  (+ ones col) [per j-chunk: 128j, 8h*65] in bf16
  c   = x (Wk_h @ rel_bias_h) [per j-chunk: 128j, 8h]  (bias term of logits)
  P^T = exp(S^T + c[j])   (no max-subtraction: logits < ~50)
  outT[65, i] accumulated over j-chunks via lhsT=V_aug, rhs=P^T; row 64 = denom
  out^T scaled by 1/denom (reciprocal_approx_fast + partition broadcast)
  y = out^T.T @ Wo + bo  (bias via broadcast bo + DVE add on PSUM copy).
"""
import numpy as np

B, N, MODEL = 4, 2048, 512
H, DK = 8, 64
SCALE = DK ** -0.5
NI = 1024          # query rows per core
NCH = MODEL // 128  # model-dim chunks
NJC = N // 128      # key/value chunks
NHP = H // 2        # head pairs
JBATCH = 4          # j-chunks per ST/PV batch

_COMPILED = None


def _build():
    import concourse.bass as bass
    from concourse import bacc
    import concourse.mybir as mybir
    import concourse.tile as tile

    F32 = mybir.dt.float32
    F32R = mybir.dt.float32r
    BF16 = mybir.dt.bfloat16
    EXP = mybir.ActivationFunctionType.Exp

    nc = bacc.Bacc("TRN2", target_bir_lowering=False, debug=False, num_devices=8)
    xt_in = nc.dram_tensor("xt", [MODEL, N], F32R, kind="ExternalInput")
    wq_in = nc.dram_tensor("wq", [MODEL, MODEL], F32R, kind="ExternalInput")
    wk_in = nc.dram_tensor("wk", [MODEL, MODEL], F32R, kind="ExternalInput")
    wv_in = nc.dram_tensor("wv", [MODEL, MODEL], F32R, kind="ExternalInput")
    wc_in = nc.dram_tensor("wc", [MODEL, H], F32R, kind="ExternalInput")
    wo_in = nc.dram_tensor("wo", [MODEL, MODEL], F32R, kind="ExternalInput")
    bo_in = nc.dram_tensor("bo", [1, MODEL], F32, kind="ExternalInput")
    onesb_in = nc.dram_tensor("onesb", [128, NJC * H], BF16, kind="ExternalInput")
    y_out = nc.dram_tensor("y", [NI, MODEL], F32, kind="ExternalOutput")

    HALF = N // 2

    with tile.TileContext(nc) as tc:
        with (
            tc.tile_pool(name="w", bufs=1) as wp,
            tc.tile_pool(name="acts", bufs=1) as ap,
            tc.tile_pool(name="big", bufs=2, space="PSUM") as ps,
            tc.tile_pool(name="pv", bufs=3, space="PSUM") as pvp,
            tc.tile_pool(name="yps", bufs=1, space="PSUM") as yp,
        ):
            # ---------- persistent tiles ----------
            wo = wp.tile([128, NCH, MODEL], F32R, tag="wo")
            bo = wp.tile([1, MODEL], F32, tag="bo")
            bo_b = wp.tile([128, MODEL], F32, tag="bo_b")
            kt_bd = ap.tile([128, H, NJC, 128], F32R, tag="kt_bd")
            qt2 = ap.tile([128, H, NI], F32R, tag="qt2")
            vv = ap.tile([128, NJC, H * 65], BF16, tag="vv")
            cc = ap.tile([128, NJC, H], F32, tag="cc")
            outt = ap.tile([128, NHP, NI], F32R, tag="outt")

            def r3(d):
                return d[:].rearrange("(c p) n -> p c n", p=128)

            nc.sync.dma_start(out=wo[:], in_=r3(wo_in))
            nc.sync.dma_start(out=bo[:], in_=bo_in[:])
            nc.gpsimd.partition_broadcast(bo_b[:], bo[:])
            # ones columns of V_aug in one strided DMA
            nc.sync.dma_start(
                out=vv[:].rearrange("p j (h e) -> p (j h) e", e=65)[:, :, 64:65],
                in_=onesb_in[:].rearrange("p (n o) -> p n o", o=1))

            with tc.tile_pool(name="s1", bufs=1) as s1:
                xt = s1.tile([128, NCH, N], F32R, tag="xt")
                zeros = s1.tile([128, NI], F32, tag="zeros")
                stage = s1.tile([128, 512], F32R, tag="stage")
                nc.vector.memset(zeros[:], 0.0)
                xsrc = r3(xt_in)
                for ch in range(NCH):
                    for half in range(2):
                        nc.sync.dma_start(
                            out=xt[:, ch, half * HALF:(half + 1) * HALF],
                            in_=xsrc[:, ch, half * HALF:(half + 1) * HALF])

                # zero-fill the off-diagonal quadrants of kt_bd
                for h in range(H):
                    nc.vector.tensor_copy(
                        kt_bd[0:64, h, :, 64:128],
                        zeros[0:64].rearrange("p (j m) -> p j m", m=64))
                    nc.vector.tensor_copy(
                        kt_bd[64:128, h, :, 0:64],
                        zeros[64:128].rearrange("p (j m) -> p j m", m=64))

                # ---- Q^T ----
                with tc.tile_pool(name="s1q", bufs=1) as s1q:
                    wq = s1q.tile([128, NCH, MODEL], F32R, tag="wq")
                    for ch in range(NCH):
                        nc.sync.dma_start(out=wq[:, ch], in_=r3(wq_in)[:, ch])
                    for hp in range(NHP):
                        q_ps = ps.tile([128, NI], F32, tag="big")
                        for ib in range(NI // 512):
                            for ch in range(NCH):
                                nc.tensor.matmul(
                                    q_ps[:, ib * 512:(ib + 1) * 512],
                                    wq[:, ch, hp * 128:(hp + 1) * 128],
                                    xt[:, ch, ib * 512:(ib + 1) * 512],
                                    start=(ch == 0), stop=(ch == NCH - 1))
                        # replicate each head's 64 dims to both partition halves
                        nc.vector.tensor_copy(qt2[0:64, 2 * hp], q_ps[0:64, :])
                        nc.vector.tensor_copy(qt2[64:128, 2 * hp + 1], q_ps[64:128, :])
                        nc.sync.dma_start(out=qt2[64:128, 2 * hp],
                                          in_=qt2[0:64, 2 * hp])
                        nc.sync.dma_start(out=qt2[0:64, 2 * hp + 1],
                                          in_=qt2[64:128, 2 * hp + 1])

                # ---- K^T (block-diagonal per head) ----
                with tc.tile_pool(name="s1k", bufs=1) as s1k:
                    wk = s1k.tile([128, NCH, MODEL], F32R, tag="wk")
                    for ch in range(NCH):
                        nc.sync.dma_start(out=wk[:, ch], in_=r3(wk_in)[:, ch])
                    for hp in range(NHP):
                        for jb in range(N // NI):
                            k_ps = ps.tile([128, NI], F32, tag="big")
                            for sb in range(NI // 512):
                                off = jb * NI + sb * 512
                                for ch in range(NCH):
                                    nc.tensor.matmul(
                                        k_ps[:, sb * 512:(sb + 1) * 512],
                                        wk[:, ch, hp * 128:(hp + 1) * 128],
                                        xt[:, ch, off:off + 512],
                                        start=(ch == 0), stop=(ch == NCH - 1))
                            # psum cols: [8 chunks x (64 even | 64 odd)]
                            kp = k_ps[:].rearrange("p (t e c) -> p t e c", t=8, e=2)
                            jcs = slice(jb * 8, jb * 8 + 8)
                            hA, hB = 2 * hp, 2 * hp + 1
                            nc.vector.tensor_copy(kt_bd[0:64, hA, jcs, 0:64],
                                                  kp[0:64, :, 0])
                            nc.vector.tensor_copy(kt_bd[64:128, hB, jcs, 64:128],
                                                  kp[64:128, :, 1])
                            # shifted halves go through SBUF staging + DMA
                            nc.vector.tensor_copy(
                                stage[0:64].rearrange("p (t c) -> p t c", t=8),
                                kp[0:64, :, 1])
                            nc.sync.dma_start(
                                out=kt_bd[64:128, hA, jcs, 64:128],
                                in_=stage[0:64].rearrange("p (t c) -> p t c", t=8))
                            nc.vector.tensor_copy(
                                stage[64:128].rearrange("p (t c) -> p t c", t=8),
                                kp[64:128, :, 0])
                            nc.sync.dma_start(
                                out=kt_bd[0:64, hB, jcs, 0:64],
                                in_=stage[64:128].rearrange("p (t c) -> p t c", t=8))

                # ---- V and c ----
                with tc.tile_pool(name="s1v", bufs=1) as s1v:
                    wv = s1v.tile([128, NCH, MODEL], F32R, tag="wv")
                    wc = s1v.tile([128, NCH, H], F32R, tag="wc")
                    for ch in range(NCH):
                        nc.sync.dma_start(out=wv[:, ch], in_=r3(wv_in)[:, ch])
                    nc.sync.dma_start(out=wc[:], in_=r3(wc_in))
                    for jc in range(NJC):
                        v_ps = ps.tile([128, NI], F32, tag="big")
                        c_ps = v_ps[:, MODEL:MODEL + 8]
                        for ch in range(NCH):
                            nc.tensor.matmul(v_ps[:, 0:MODEL],
                                             xt[:, ch, jc * 128:(jc + 1) * 128],
                                             wv[:, ch],
                                             start=(ch == 0), stop=(ch == NCH - 1))
                            nc.tensor.matmul(c_ps,
                                             xt[:, ch, jc * 128:(jc + 1) * 128],
                                             wc[:, ch],
                                             start=(ch == 0), stop=(ch == NCH - 1))
                        for h in range(H):
                            nc.vector.tensor_copy(vv[:, jc, h * 65:h * 65 + 64],
                                                  v_ps[:, h * 64:(h + 1) * 64])
                        nc.vector.tensor_copy(cc[:, jc], c_ps)

            # ---------- stage 2: attention ----------
            with (
                tc.tile_pool(name="pt", bufs=2 * JBATCH) as ptp,
                tc.tile_pool(name="norm", bufs=4) as np_,
                tc.tile_pool(name="ysb", bufs=3) as yp_sb,
            ):
                for h in range(H):
                    hp, hr = h // 2, (h % 2) * 64
                    pv_a = pvp.tile([65, 512], F32, tag="pv")
                    pv_b = pvp.tile([65, 512], F32, tag="pv")
                    pv_tiles = [pv_a, pv_b]
                    for jc0 in range(0, NJC, JBATCH):
                        pts = []
                        st_list = []
                        for jc in range(jc0, jc0 + JBATCH):
                            st_ps = ps.tile([128, NI], F32, tag="big")
                            st_list.append(st_ps)
                            for ih in range(2):
                                nc.tensor.matmul(
                                    st_ps[:, ih * 512:(ih + 1) * 512],
                                    kt_bd[:, h, jc],
                                    qt2[:, h, ih * 512:(ih + 1) * 512],
                                    start=True, stop=True)
                        for k, jc in enumerate(range(jc0, jc0 + JBATCH)):
                            pt = ptp.tile([128, NI], BF16, tag="pt")
                            pts.append(pt)
                            nc.scalar.activation(pt[:], st_list[k][:], EXP,
                                                 bias=cc[:, jc, h:h + 1], scale=1.0)
                        for k, jc in enumerate(range(jc0, jc0 + JBATCH)):
                            for ih in range(2):
                                nc.tensor.matmul(
                                    pv_tiles[ih][:],
                                    vv[:, jc, h * 65:(h + 1) * 65],
                                    pts[k][:, ih * 512:(ih + 1) * 512],
                                    start=(jc == 0), stop=(jc == NJC - 1))
                    for ih in range(2):
                        den = np_.tile([1, 512], F32, tag="den")
                        nc.vector.tensor_copy(den[:], pv_tiles[ih][64:65, :])
                        rrow = np_.tile([1, 512], F32, tag="rrow")
                        nc.vector.reciprocal_approx_fast(rrow[:], den[:])
                        rb = np_.tile([64, 512], F32, tag="rb")
                        nc.gpsimd.partition_broadcast(rb[:], rrow[:])
                        nc.vector.tensor_tensor(
                            out=outt[hr:hr + 64, hp, ih * 512:(ih + 1) * 512],
                            in0=pv_tiles[ih][0:64, :], in1=rb[:],
                            op=mybir.AluOpType.mult)

                # ---------- stage 3: output projection ----------
                for ib in range(NI // 128):
                    y_ps = yp.tile([128, MODEL], F32, tag="y")
                    for ch in range(NCH):
                        nc.tensor.matmul(y_ps[:],
                                         outt[:, ch, ib * 128:(ib + 1) * 128],
                                         wo[:, ch],
                                         start=(ch == 0), stop=(ch == NCH - 1))
                    y_sb = yp_sb.tile([128, MODEL], F32, tag="ysb")
                    nc.vector.tensor_tensor(out=y_sb[:], in0=y_ps[:], in1=bo_b[:],
                                            op=mybir.AluOpType.add)
                    nc.sync.dma_start(out=y_out[ib * 128:(ib + 1) * 128, :],
                                      in_=y_sb[:])

    nc.compile()
    return nc


def _get_compiled():
    global _COMPILED
    if _COMPILED is None:
        _COMPILED = _build()
    return _COMPILED


def kernel(x, Wq, Wk, Wv, Wo, bo, rel_content_bias, _trace=False):
    from concourse.bass_utils import run_bass_kernel_spmd
    import ml_dtypes

    nc = _get_compiled()

    x = np.asarray(x, dtype=np.float32)
    Wq = np.asarray(Wq, dtype=np.float32)
    Wk = np.asarray(Wk, dtype=np.float32)
    Wv = np.asarray(Wv, dtype=np.float32)
    Wo = np.asarray(Wo, dtype=np.float32)
    bo = np.asarray(bo, dtype=np.float32)
    bias = np.asarray(rel_content_bias, dtype=np.float32).reshape(H, DK)

    Wq_s = (Wq * SCALE).astype(np.float32)
    # c[j, h] = k_j . bias_h  ->  x @ (Wk_h @ bias_h)
    Wc = np.einsum("mhd,hd->mh", Wk.reshape(MODEL, H, DK), bias).astype(np.float32)
    onesb = np.ones((128, NJC * H), ml_dtypes.bfloat16)
    shared = {"wq": Wq_s, "wk": Wk, "wv": Wv, "wc": Wc, "wo": Wo,
              "bo": bo[None, :], "onesb": onesb}

    in_maps = []
    for c in range(8):
        b, half = c // 2, c % 2
        xt = np.ascontiguousarray(x[b].T)              # [512, 2048]
        if half:
            xt = np.ascontiguousarray(np.roll(xt, -NI, axis=1))
        in_maps.append({"xt": xt, **shared})

    res = run_bass_kernel_spmd(nc, in_maps, core_ids=list(range(8)),
                               trace=_trace)
    out = np.empty((B, N, MODEL), np.float32)
    for c in range(8):
        b, half = c // 2, c % 2
        out[b, half * NI:(half + 1) * NI, :] = res.results[c]["y"]
    if _trace:
        return out, res
    return out


# revision 10
# speedup vs baseline: 1.0535x; 1.0535x over previous
"""Trainium2 Bass kernel for multi-head attention (b=4, n=2048, d=512, h=8, dk=dv=64).

Sharding: 8 cores = 4 batches x 2 query-halves. Each core computes K/V for its
full batch sequence (2048) and attention outputs for its 1024 query rows.
No collectives needed; host stacks the per-core [1024, 512] outputs.

Per-core dataflow (f32r = TF32-like fast fp32 matmul mode; PV in bf16):
  x^T [512, 2048] staged in SBUF.
  Q^T per head, replicated to both partition halves: qt2 [128, h, i]
  K^T per head stored block-diagonally per j-chunk: kt_bd [128, h, jc, 128]
    rows 0:64  = K^T dims x even 64-j half   (cols 0:64)
    rows 64:128= K^T dims x odd 64-j half    (cols 64:128), zeros elsewhere
  -> S^T matmul has K=128 (full PE rate): out rows = 128 consecutive j's.
  V   = x Wv  (+ ones col) [per j-chunk: 128j, 8h*65] in bf16
  c   = x (Wk_h @ rel_bias_h) [per j-chunk: 128j, 8h]  (bias term of logits)
  P^T = exp(S^T + c[j])   (no max-subtraction: logits < ~50)
  outT[65, i] accumulated over j-chunks via lhsT=V_aug, rhs=P^T; row 64 = denom
  out^T scaled by 1/denom (reciprocal_approx_fast + partition broadcast)
  y = out^T.T @ Wo + bo  (bias via broadcast bo + DVE add on PSUM copy).
"""
import numpy as np

B, N, MODEL = 4, 2048, 512
H, DK = 8, 64
SCALE = DK ** -0.5
NI = 1024          # query rows per core
NCH = MODEL // 128  # model-dim chunks
NJC = N // 128      # key/value chunks
NHP = H // 2        # head pairs
JBATCH = 4          # j-chunks per ST/PV batch

_COMPILED = None


def _build():
    import concourse.bass as bass
    from concourse import bacc
    import concourse.mybir as mybir
    import concourse.tile as tile

    F32 = mybir.dt.float32
    F32R = mybir.dt.float32r
    BF16 = mybir.dt.bfloat16
    EXP = mybir.ActivationFunctionType.Exp

    nc = bacc.Bacc("TRN2", target_bir_lowering=False, debug=False, num_devices=8)
    xt_in = nc.dram_tensor("xt", [MODEL, N], F32R, kind="ExternalInput")
    wq_in = nc.dram_tensor("wq", [MODEL, H * 128], F32R, kind="ExternalInput")
    wk_in = nc.dram_tensor("wk", [MODEL, H * 128], F32R, kind="ExternalInput")
    wv_in = nc.dram_tensor("wv", [MODEL, MODEL], F32R, kind="ExternalInput")
    wc_in = nc.dram_tensor("wc", [MODEL, H], F32R, kind="ExternalInput")
    wo_in = nc.dram_tensor("wo", [MODEL, MODEL], F32R, kind="ExternalInput")
    bo_in = nc.dram_tensor("bo", [1, MODEL], F32, kind="ExternalInput")
    onesb_in = nc.dram_tensor("onesb", [128, NJC * H], BF16, kind="ExternalInput")
    y_out = nc.dram_tensor("y", [NI, MODEL], F32, kind="ExternalOutput")

    HALF = N // 2

    with tile.TileContext(nc) as tc:
        with (
            tc.tile_pool(name="w", bufs=1) as wp,
            tc.tile_pool(name="acts", bufs=1) as ap,
            tc.tile_pool(name="big", bufs=2, space="PSUM") as ps,
            tc.tile_pool(name="pv", bufs=3, space="PSUM") as pvp,
            tc.tile_pool(name="yps", bufs=1, space="PSUM") as yp,
        ):
            # ---------- persistent tiles ----------
            wo = wp.tile([128, NCH, MODEL], F32R, tag="wo")
            bo = wp.tile([1, MODEL], F32, tag="bo")
            bo_b = wp.tile([128, MODEL], F32, tag="bo_b")
            kt_bd = ap.tile([128, H, NJC, 128], F32R, tag="kt_bd")
            qt2 = ap.tile([128, H, NI], F32R, tag="qt2")
            vv = ap.tile([128, NJC, H * 65], BF16, tag="vv")
            cc = ap.tile([128, NJC, H], F32, tag="cc")
            outt = ap.tile([128, NHP, NI], F32R, tag="outt")

            def r3(d):
                return d[:].rearrange("(c p) n -> p c n", p=128)

            nc.sync.dma_start(out=wo[:], in_=r3(wo_in))
            nc.sync.dma_start(out=bo[:], in_=bo_in[:])
            nc.gpsimd.partition_broadcast(bo_b[:], bo[:])
            # ones columns of V_aug in one strided DMA
            nc.sync.dma_start(
                out=vv[:].rearrange("p j (h e) -> p (j h) e", e=65)[:, :, 64:65],
                in_=onesb_in[:].rearrange("p (n o) -> p n o", o=1))

            with tc.tile_pool(name="s1", bufs=1) as s1:
                xt = s1.tile([128, NCH, N], F32R, tag="xt")
                zeros = s1.tile([128, 512], F32, tag="zeros")
                nc.vector.memset(zeros[:], 0.0)
                xsrc = r3(xt_in)
                for ch in range(NCH):
                    for half in range(2):
                        nc.sync.dma_start(
                            out=xt[:, ch, half * HALF:(half + 1) * HALF],
                            in_=xsrc[:, ch, half * HALF:(half + 1) * HALF])

                # zero-fill the off-diagonal quadrants of kt_bd
                for h in range(H):
                    for jh in range(2):
                        js = slice(jh * 8, jh * 8 + 8)
                        nc.vector.tensor_copy(
                            kt_bd[0:64, h, js, 64:128],
                            zeros[0:64].rearrange("p (j m) -> p j m", m=64))
                        nc.vector.tensor_copy(
                            kt_bd[64:128, h, js, 0:64],
                            zeros[64:128].rearrange("p (j m) -> p j m", m=64))

                # ---- Q^T ----
                with tc.tile_pool(name="s1q", bufs=1) as s1q:
                    wq = s1q.tile([128, NCH, H * 128], F32R, tag="wq")
                    for ch in range(NCH):
                        nc.sync.dma_start(out=wq[:, ch], in_=r3(wq_in)[:, ch])
                    for h in range(H):
                        q_ps = ps.tile([128, NI], F32, tag="big")
                        for ib in range(NI // 512):
                            for ch in range(NCH):
                                nc.tensor.matmul(
                                    q_ps[:, ib * 512:(ib + 1) * 512],
                                    wq[:, ch, h * 128:(h + 1) * 128],
                                    xt[:, ch, ib * 512:(ib + 1) * 512],
                                    start=(ch == 0), stop=(ch == NCH - 1))
                        nc.vector.tensor_copy(qt2[:, h], q_ps[:])

                # ---- K^T (block-diagonal per head) ----
                with tc.tile_pool(name="s1k", bufs=1) as s1k:
                    wk = s1k.tile([128, NCH, H * 128], F32R, tag="wk")
                    for ch in range(NCH):
                        nc.sync.dma_start(out=wk[:, ch], in_=r3(wk_in)[:, ch])
                    for h in range(H):
                        for jb in range(N // NI):
                            k_ps = ps.tile([128, NI], F32, tag="big")
                            for sb in range(NI // 512):
                                off = jb * NI + sb * 512
                                for ch in range(NCH):
                                    nc.tensor.matmul(
                                        k_ps[:, sb * 512:(sb + 1) * 512],
                                        wk[:, ch, h * 128:(h + 1) * 128],
                                        xt[:, ch, off:off + 512],
                                        start=(ch == 0), stop=(ch == NCH - 1))
                            # psum rows 0:64 and 64:128 both hold K^T_h dims;
                            # cols: [8 chunks x (64 even | 64 odd)]
                            kp = k_ps[:].rearrange("p (t e c) -> p t e c", t=8, e=2)
                            jcs = slice(jb * 8, jb * 8 + 8)
                            nc.vector.tensor_copy(kt_bd[0:64, h, jcs, 0:64],
                                                  kp[0:64, :, 0])
                            nc.vector.tensor_copy(kt_bd[64:128, h, jcs, 64:128],
                                                  kp[64:128, :, 1])

                # ---- V and c ----
                with tc.tile_pool(name="s1v", bufs=1) as s1v:
                    wv = s1v.tile([128, NCH, MODEL], F32R, tag="wv")
                    wc = s1v.tile([128, NCH, H], F32R, tag="wc")
                    for ch in range(NCH):
                        nc.sync.dma_start(out=wv[:, ch], in_=r3(wv_in)[:, ch])
                    nc.sync.dma_start(out=wc[:], in_=r3(wc_in))
                    for jc in range(NJC):
                        v_ps = ps.tile([128, NI], F32, tag="big")
                        c_ps = v_ps[:, MODEL:MODEL + 8]
                        for ch in range(NCH):
                            nc.tensor.matmul(v_ps[:, 0:MODEL],
                                             xt[:, ch, jc * 128:(jc + 1) * 128],
                                             wv[:, ch],
                                             start=(ch == 0), stop=(ch == NCH - 1))
                            nc.tensor.matmul(c_ps,
                                             xt[:, ch, jc * 128:(jc + 1) * 128],
                                             wc[:, ch],
                                             start=(ch == 0), stop=(ch == NCH - 1))
                        for h in range(H):
                            nc.vector.tensor_copy(vv[:, jc, h * 65:h * 65 + 64],
                                                  v_ps[:, h * 64:(h + 1) * 64])
                        nc.vector.tensor_copy(cc[:, jc], c_ps)

            # ---------- stage 2: attention ----------
            with (
                tc.tile_pool(name="pt", bufs=2 * JBATCH) as ptp,
                tc.tile_pool(name="norm", bufs=4) as np_,
                tc.tile_pool(name="ysb", bufs=3) as yp_sb,
            ):
                for h in range(H):
                    hp, hr = h // 2, (h % 2) * 64
                    pv_a = pvp.tile([65, 512], F32, tag="pv")
                    pv_b = pvp.tile([65, 512], F32, tag="pv")
                    pv_tiles = [pv_a, pv_b]
                    for jc0 in range(0, NJC, JBATCH):
                        pts = []
                        st_list = []
                        for jc in range(jc0, jc0 + JBATCH):
                            st_ps = ps.tile([128, NI], F32, tag="big")
                            st_list.append(st_ps)
                            for ih in range(2):
                                nc.tensor.matmul(
                                    st_ps[:, ih * 512:(ih + 1) * 512],
                                    kt_bd[:, h, jc],
                                    qt2[:, h, ih * 512:(ih + 1) * 512],
                                    start=True, stop=True)
                        for k, jc in enumerate(range(jc0, jc0 + JBATCH)):
                            pt = ptp.tile([128, NI], BF16, tag="pt")
                            pts.append(pt)
                            nc.scalar.activation(pt[:], st_list[k][:], EXP,
                                                 bias=cc[:, jc, h:h + 1], scale=1.0)
                        for k, jc in enumerate(range(jc0, jc0 + JBATCH)):
                            for ih in range(2):
                                nc.tensor.matmul(
                                    pv_tiles[ih][:],
                                    vv[:, jc, h * 65:(h + 1) * 65],
                                    pts[k][:, ih * 512:(ih + 1) * 512],
                                    start=(jc == 0), stop=(jc == NJC - 1))
                    for ih in range(2):
                        den = np_.tile([1, 512], F32, tag="den")
                        nc.vector.tensor_copy(den[:], pv_tiles[ih][64:65, :])
                        rrow = np_.tile([1, 512], F32, tag="rrow")
                        nc.vector.reciprocal_approx_fast(rrow[:], den[:])
                        rb = np_.tile([64, 512], F32, tag="rb")
                        nc.gpsimd.partition_broadcast(rb[:], rrow[:])
                        nc.vector.tensor_tensor(
                            out=outt[hr:hr + 64, hp, ih * 512:(ih + 1) * 512],
                            in0=pv_tiles[ih][0:64, :], in1=rb[:],
                            op=mybir.AluOpType.mult)

                # ---------- stage 3: output projection ----------
                for ib in range(NI // 128):
                    y_ps = yp.tile([128, MODEL], F32, tag="y")
                    for ch in range(NCH):
                        nc.tensor.matmul(y_ps[:],
                                         outt[:, ch, ib * 128:(ib + 1) * 128],
                                         wo[:, ch],
                                         start=(ch == 0), stop=(ch == NCH - 1))
                    y_sb = yp_sb.tile([128, MODEL], F32, tag="ysb")
                    nc.vector.tensor_tensor(out=y_sb[:], in0=y_ps[:], in1=bo_b[:],
                                            op=mybir.AluOpType.add)
                    nc.sync.dma_start(out=y_out[ib * 128:(ib + 1) * 128, :],
                                      in_=y_sb[:])

    nc.compile()
    return nc


def _get_compiled():
    global _COMPILED
    if _COMPILED is None:
        _COMPILED = _build()
    return _COMPILED


def kernel(x, Wq, Wk, Wv, Wo, bo, rel_content_bias, _trace=False):
    from concourse.bass_utils import run_bass_kernel_spmd
    import ml_dtypes

    nc = _get_compiled()

    x = np.asarray(x, dtype=np.float32)
    Wq = np.asarray(Wq, dtype=np.float32)
    Wk = np.asarray(Wk, dtype=np.float32)
    Wv = np.asarray(Wv, dtype=np.float32)
    Wo = np.asarray(Wo, dtype=np.float32)
    bo = np.asarray(bo, dtype=np.float32)
    bias = np.asarray(rel_content_bias, dtype=np.float32).reshape(H, DK)

    Wq_s = (Wq * SCALE).astype(np.float32)
    def rep2(w):  # [512, h*64] -> [512, h*128] with each head's 64 cols doubled
        w3 = w.reshape(MODEL, H, DK)
        return np.concatenate([w3, w3], axis=2).reshape(MODEL, H * 128)
    Wq_s = rep2(Wq_s)
    # c[j, h] = k_j . bias_h  ->  x @ (Wk_h @ bias_h)
    Wc = np.einsum("mhd,hd->mh", Wk.reshape(MODEL, H, DK), bias).astype(np.float32)
    onesb = np.ones((128, NJC * H), ml_dtypes.bfloat16)
    Wk_r = rep2(Wk)
    shared = {"wq": Wq_s, "wk": Wk_r, "wv": Wv, "wc": Wc, "wo": Wo,
              "bo": bo[None, :], "onesb": onesb}

    in_maps = []
    for c in range(8):
        b, half = c // 2, c % 2
        xt = np.ascontiguousarray(x[b].T)              # [512, 2048]
        if half:
            xt = np.ascontiguousarray(np.roll(xt, -NI, axis=1))
        in_maps.append({"xt": xt, **shared})

    res = run_bass_kernel_spmd(nc, in_maps, core_ids=list(range(8)),
                               trace=_trace)
    out = np.empty((B, N, MODEL), np.float32)
    for c in range(8):
        b, half = c // 2, c % 2
        out[b, half * NI:(half + 1) * NI, :] = res.results[c]["y"]
    if _trace:
        return out, res
    return out
